# revision 8
# baseline (speedup 1.0000x reference)
"""GAT (2-layer, PyG-style) Trainium2 Bass kernel, 8-core SPMD — v2.

Strategy (edge parallelism by destination):
  - Add self loops, sort edges by dst, partition dst-node blocks of 128
    across 8 cores (contiguous block ranges).
  - Per layer, a node-feature table T (row-per-node: [h | a_src], bf16 h
    + f32 attn scalars, 256B-multiple row stride) is built on device
    (distributed across cores), assembled on host, and re-fed replicated
    to every core.
  - Edge phase per core, per dst-block "slot": batched dma_gather of
    T[src] rows, a host-precomputed one-hot ST table (u8, cast to bf16
    during the DMA) giving ST[d, e] = 1[dst_e == d], then per chunk of
    G=8 128-edge tiles:
      S[e,(t,d)] = (iota == dstloc)               (one DVE is_equal)
      uE[e,h]    = ST_t^T(d,e) @ a_dst_block      (PE matmul per tile)
      u = uE + a_src_gathered                     (one DVE add)
      p = exp(leaky_relu(u))                      (one DVE stt + one ACT)
      M = h_gathered * p (4D broadcast)           (one DVE mult)
      acc[d,:] += S_t^T @ [M | p]                 (PE matmul per tile)
    Segment softmax without max-subtraction (logits are O(10), exact in
    f32: softmax is shift-invariant so this matches the reference).
  - Epilogues batched over SG=4 slots: out = acc[:, :HC]/acc[:, HC:]
    (per head), + bias, ELU (layer 1) or head-mean (layer 2).
  - Node phases avoid transposes: aa = x @ (W @ A_blockdiag) with the
    product precomputed on host; xT comes from one DMA transpose.

Three launches (host assembles/replicates tables between them; that
host work is pure data movement, all math is on device):
  A0: x @ W1, x @ WA1 -> per-node h1/as1/ad1 slice            (distributed)
  A1: layer-1 edge phase (h1' kept in SBUF); h1' @ W2 / @ WA2 -> t2 slice
  B1: layer-2 edge phase -> final out slice
"""

import sys

sys.path.insert(0, "/opt/trn_rl_repo")

import math
import numpy as np
import ml_dtypes

import concourse.bass as bass
import concourse.bacc as bacc
import concourse.tile as tile
from concourse import mybir
from concourse.bass_utils import run_bass_kernel_spmd
from concourse.masks import make_identity

BF16 = ml_dtypes.bfloat16
FP8 = ml_dtypes.float8_e4m3
F32 = mybir.dt.float32
BF = mybir.dt.bfloat16
I16 = mybir.dt.int16
E4 = mybir.dt.float8e4

P = 128
NCORES = 8
SPLIT = 32768
NEG_SLOPE = 0.2
PAD_DST = 1000.0  # dstloc sentinel: matches no d in [0,128)
G = 8   # 128-edge tiles per compute chunk (= max tiles per dma_gather)
SG = 4  # slots per batched epilogue group


def _cfg(N, E, IN, H1, C1, H2, C2):
    nblk = math.ceil(N / P)
    slots = math.ceil(nblk / NCORES)
    return dict(
        N=N, E=E, IN=IN, H1=H1, C1=C1, H2=H2, C2=C2,
        D1=H1 * C1, D2=H2 * C2,
        NBLK=nblk, SLOTS=slots, NPC=slots * P, NPAD=nblk * P,
        # table row lengths in bf16 elems (256B-multiple strides)
        ROW1=_row_elems(H1 * C1 + 2 * 2 * H1),  # h bf16 + as,ad f32
        ROW2=_row_elems(H2 * C2 + 2 * 2 * H2),
    )


def _row_elems(used_bf16_elems):
    return ((used_bf16_elems + 127) // 128) * 128


CFG = _cfg(N=50000, E=800000, IN=128, H1=4, C1=32, H2=8, C2=32)

IOTA_TILE = np.tile(np.arange(P, dtype=np.float32)[None, :],
                    (P, G)).astype(BF16)  # [P, G*P]: iota over d per tile


# ---------------------------------------------------------------------------
# Host-side edge plan
# ---------------------------------------------------------------------------

def build_edge_plan(cfg, src, dst):
    """Sort by dst, bucket into (core, slot) dst blocks, split each block's
    edges by src < SPLIT, pad each group to a multiple of 128.

    Returns a static `plan` (identical across cores) plus per-core data
    buffers (gather indices, dstloc per tile, one-hot ST table)."""
    slots, nblk = cfg["SLOTS"], cfg["NBLK"]
    order = np.argsort(dst, kind="stable")
    ss = src[order].astype(np.int64)
    dd = dst[order].astype(np.int64)
    blk_edges = {}
    bounds = np.searchsorted(dd, np.arange(nblk + 1) * P)
    for b in range(nblk):
        lo, hi = bounds[b], bounds[b + 1]
        s_b, d_b = ss[lo:hi], dd[lo:hi]
        a_mask = s_b < SPLIT
        blk_edges[b] = (
            (s_b[a_mask], d_b[a_mask] - b * P),
            (s_b[~a_mask] - SPLIT, d_b[~a_mask] - b * P),
        )

    # balance: sort blocks by edge count desc, hand out 8 per slot (one per
    # core) so the per-slot max over cores stays close to the mean
    counts = bounds[1:] - bounds[:-1]
    border = np.argsort(-counts, kind="stable")
    blk_of = np.full((NCORES, slots), -1, np.int64)
    for s in range(slots):
        grp = border[s * NCORES:(s + 1) * NCORES]
        for c, b in enumerate(grp):
            blk_of[c, s] = b

    # static per-slot tile counts (max over cores)
    TA, TB = [], []
    for s in range(slots):
        mxa = mxb = 0
        for c in range(NCORES):
            b = blk_of[c, s]
            if b >= 0:
                mxa = max(mxa, len(blk_edges[b][0][0]))
                mxb = max(mxb, len(blk_edges[b][1][0]))
        ta = max(1, math.ceil(mxa / P))  # >=1 so PSUM is always written
        tb = math.ceil(mxb / P)
        TA.append(ta)
        TB.append(tb)

    # gather call descriptors: (slot, group, tile_offset_in_slot, ntiles)
    # HW cap: a single dma_gather crashes beyond 1024 indices -> <=8 tiles
    MAX_NT = G
    calls = []
    ttot = 0
    tile_off = []  # per slot, global tile offset
    for s in range(slots):
        tile_off.append(ttot)
        for grp, t0, T in ((0, 0, TA[s]), (1, TA[s], TB[s])):
            off = 0
            while off < T:
                nt = min(MAX_NT, T - off)
                calls.append((s, grp, t0 + off, nt))
                off += nt
        ttot += TA[s] + TB[s]
    ncalls = len(calls)

    # per-core buffers (laid out per (slot, group); gather-call chunking
    # slices this layout at tile boundaries, which lines up exactly)
    Lg = ttot * (P // 16)
    gidx = np.full((NCORES, 16, Lg), -1, np.int16)
    dstloc = np.full((NCORES, P, ttot), PAD_DST, np.float32)
    for c in range(NCORES):
        for s in range(slots):
            b = blk_of[c, s]
            for grp, t0, T in ((0, 0, TA[s]), (1, TA[s], TB[s])):
                if T == 0:
                    continue
                idx_arr = np.zeros(T * P, np.int64)  # pad rows gather row 0
                if b >= 0:
                    sg, dg = blk_edges[b][grp]
                else:
                    sg = dg = np.zeros(0, np.int64)
                n = len(sg)
                assert n <= T * P
                if n:
                    idx_arr[:n] = sg
                    g0 = tile_off[s] + t0
                    pos = np.arange(n)
                    dstloc[c, pos % P, g0 + pos // P] = dg
                col0 = (tile_off[s] + t0) * (P // 16)
                gidx[c, :, col0:col0 + T * (P // 16)] = (
                    idx_arr.reshape(T * (P // 16), 16).T.astype(np.int16)
                )

    # fp8 one-hot tables (1.0 = 0x38 in e4m3):
    #   stf8[c, d, t*128+e] = 1 iff dstloc[c, e, t] == d   (lhsT for uE)
    #   sf8[c, e, t*128+d]  = 1 iff dstloc[c, e, t] == d   (lhsT for acc)
    ONE = np.float32(1.0).astype(FP8).view(np.uint8)
    stf8 = np.zeros((NCORES, P, ttot * P), np.uint8)
    sf8 = np.zeros((NCORES, P, ttot * P), np.uint8)
    cc, ee, tt = np.nonzero(dstloc < P)
    vv = dstloc[cc, ee, tt].astype(np.int64)
    stf8[cc, vv, tt * P + ee] = ONE
    sf8[cc, ee, tt * P + vv] = ONE

    slot_tiles = [(TA[s], TB[s]) for s in range(slots)]
    plan = dict(calls=calls, slot_tiles=slot_tiles, tile_off=tile_off,
                ttot=ttot, ncalls=ncalls, Lg=Lg,
                Tmax=max(a + b for a, b in slot_tiles), blk_of=blk_of)
    data = dict(
        gidx=np.tile(gidx, (1, 8, 1)),          # [NC, 128, Lg]
        sf8=sf8.view(FP8),                       # [NC, 128, ttot*128] fp8
        stf8=stf8.view(FP8),                     # [NC, 128, ttot*128] fp8
    )
    return plan, data


# ---------------------------------------------------------------------------
# Bass program builders
# ---------------------------------------------------------------------------

def build_A0(cfg):
    """Distributed phase-0 of layer 1: t1s = [h1 | as1 | ad1] for own nodes.
    h = x @ W1 (bf16), aa = x @ (W1 @ A1_blockdiag) (f32 out columns)."""
    NPC, IN, D1, H1 = cfg["NPC"], cfg["IN"], cfg["D1"], cfg["H1"]
    slots = cfg["SLOTS"]
    OC = D1 + 4 * H1  # out row in bf16 elems: D1 bf16 + 2*H1 f32
    nc = bacc.Bacc("TRN2", target_bir_lowering=False, debug=False)
    xs = nc.declare_dram_parameter("xs", [NPC, IN], F32, isOutput=False)
    W1 = nc.declare_dram_parameter("W1", [IN, D1], BF, isOutput=False)
    WA1 = nc.declare_dram_parameter("WA1", [IN, 2 * H1], BF, isOutput=False)
    xbf = nc.declare_dram_parameter("xbf", [NPC, IN], BF, isOutput=True)
    t1s = nc.declare_dram_parameter("t1s", [NPC, OC], BF, isOutput=True)

    with tile.TileContext(nc) as tc:
        with tc.tile_pool(name="const", bufs=1) as cp, \
             tc.tile_pool(name="work", bufs=3) as wp, \
             tc.tile_pool(name="psum", bufs=2, space="PSUM") as pp:
            w1 = cp.tile([IN, D1], BF, tag="w1")
            nc.sync.dma_start(out=w1[:], in_=W1[:])
            wa1 = cp.tile([IN, 2 * H1], BF, tag="wa1")
            nc.sync.dma_start(out=wa1[:], in_=WA1[:])
            # cast x to bf16 in DRAM, then chunked transposed loads
            xT = cp.tile([IN, NPC], BF, tag="xT")
            stage = cp.tile([P, slots * OC], BF, tag="stage")
            NCH = 7
            cuts = [round(slots * k / NCH) for k in range(NCH + 1)]
            for k in range(NCH):
                r = slice(cuts[k] * P, cuts[k + 1] * P)
                nc.gpsimd.dma_start(out=xbf[r, :], in_=xs[r, :])
                nc.sync.dma_start(out=xT[:, r], in_=xbf[r, :],
                                  transpose=True)

            for k in range(NCH):
                for s in range(cuts[k], cuts[k + 1]):
                    xTs = xT[:, s * P:(s + 1) * P]
                    hp = pp.tile([P, D1], F32, tag="hp")
                    nc.tensor.matmul(out=hp[:], lhsT=xTs, rhs=w1[:],
                                     start=True, stop=True)
                    nc.vector.tensor_copy(
                        out=stage[:, s * OC:s * OC + D1], in_=hp[:])
                    aap = pp.tile([P, 2 * H1], F32, tag="aap")
                    nc.tensor.matmul(out=aap[:], lhsT=xTs, rhs=wa1[:],
                                     start=True, stop=True)
                    nc.scalar.copy(
                        out=stage[:, s * OC + D1:(s + 1) * OC].bitcast(F32),
                        in_=aap[:])
                sc = slice(cuts[k], cuts[k + 1])
                nc.sync.dma_start(
                    out=t1s.rearrange("(s p) c -> p s c", p=P)[:, sc, :],
                    in_=stage[:].rearrange("p (s c) -> p s c", c=OC)[:, sc, :])
    nc.compile()
    return nc


def _edge_phase(nc, tc, cfg, plan, layer, T_dram, adown, gidx_d,
                s8_d, st8_d, bbc_d, store_cb):
    """Shared edge phase. store_cb(s0, ng, out_sb) is called per epilogue
    group with the batched [P, ng*HC] (layer 1, post-ELU bf16 written
    already) — actually store_cb(s, ...) abstracts per-layer epilogue
    output handling; see callers."""
    H = cfg["H1"] if layer == 1 else cfg["H2"]
    HC = cfg["D1"] if layer == 1 else cfg["D2"]
    ROW = cfg["ROW1"] if layer == 1 else cfg["ROW2"]
    NPAD = cfg["NPAD"]
    slots = cfg["SLOTS"]
    ttot, Lg, Tmax = plan["ttot"], plan["Lg"], plan["Tmax"]
    C = HC // H
    W = HC + H  # acc row width

    cp = tc.alloc_tile_pool(name="ec", bufs=1)
    gp = tc.alloc_tile_pool(name="gb", bufs=3)
    stp = tc.alloc_tile_pool(name="st", bufs=3)
    sp = tc.alloc_tile_pool(name="es", bufs=3)
    pp = tc.alloc_tile_pool(name="eps", bufs=2, space="PSUM")
    ap = tc.alloc_tile_pool(name="eacc", bufs=2, space="PSUM")
    ep = tc.alloc_tile_pool(name="epi", bufs=2)

    gidx = cp.tile([P, Lg], I16, tag="gidx")
    nc.sync.dma_start(out=gidx[:], in_=gidx_d[:])
    ado = cp.tile([P, slots * H], F32, tag="ado")
    nc.sync.dma_start(out=ado[:], in_=adown[:])
    bbc = cp.tile([P, bbc_d.shape[1]], F32, tag="bbc")
    nc.sync.dma_start(out=bbc[:], in_=bbc_d[:])

    calls_by_slot = {}
    for (s, grp, toff, nt) in plan["calls"]:
        calls_by_slot.setdefault(s, []).append((grp, toff, nt))

    accbuf = None
    group = []  # (s, acc_tile) pending epilogue

    def flush_group():
        nonlocal accbuf, group
        if not group:
            return
        ng = len(group)
        s0 = group[0][0]
        # batched epilogue over ng slots from accbuf [P, ng*W] f32
        a3 = accbuf[:, 0:ng * W].rearrange("p (s w) -> p s w", w=W)
        rs = ep.tile([P, SG * H], F32, tag="rs")
        nc.vector.reciprocal(out=rs[:, 0:ng * H], in_=a3[:, :, HC:HC + H])
        on = ep.tile([P, SG * HC], F32, tag="on")
        if layer == 1:
            nc.vector.tensor_tensor(
                out=on[:, 0:ng * HC].rearrange(
                    "p (s h c) -> p s h c", h=H, c=C),
                in0=a3[:, :, 0:HC].rearrange("p s (h c) -> p s h c", h=H),
                in1=rs[:, 0:ng * H].rearrange(
                    "p (s h) -> p s h", h=H).to_broadcast([P, ng, H, C]),
                op=mybir.AluOpType.mult)
            ob = ep.tile([P, SG * HC], F32, tag="ob")
            nc.vector.tensor_tensor(out=ob[:, 0:ng * HC],
                                    in0=on[:, 0:ng * HC],
                                    in1=bbc[:, 0:ng * HC],
                                    op=mybir.AluOpType.add)
            # ELU = relu(x) + exp(min(x,0)) - 1
            tmin = ep.tile([P, SG * HC], F32, tag="tmin")
            nc.vector.tensor_scalar_min(out=tmin[:, 0:ng * HC],
                                        in0=ob[:, 0:ng * HC], scalar1=0.0)
            ex = ep.tile([P, SG * HC], F32, tag="ex")
            nc.scalar.activation(out=ex[:, 0:ng * HC],
                                 in_=tmin[:, 0:ng * HC],
                                 func=mybir.ActivationFunctionType.Exp)
            rl = ep.tile([P, SG * HC], F32, tag="rl")
            nc.vector.tensor_scalar_max(out=rl[:, 0:ng * HC],
                                        in0=ob[:, 0:ng * HC], scalar1=0.0)
            store_cb(s0, ng, ex, rl)
        else:
            C2 = C
            rs8 = ep.tile([P, SG * H], F32, tag="rs8")
            nc.vector.tensor_scalar_mul(out=rs8[:, 0:ng * H],
                                        in0=rs[:, 0:ng * H], scalar1=1.0 / H)
            nc.vector.tensor_tensor(
                out=on[:, 0:ng * HC].rearrange(
                    "p (s h c) -> p s h c", h=H, c=C),
                in0=a3[:, :, 0:HC].rearrange("p s (h c) -> p s h c", h=H),
                in1=rs8[:, 0:ng * H].rearrange(
                    "p (s h) -> p s h", h=H).to_broadcast([P, ng, H, C]),
                op=mybir.AluOpType.mult)
            red = ep.tile([P, SG * C2], F32, tag="red")
            nc.vector.reduce_sum(
                out=red[:, 0:ng * C2].rearrange("p (s c) -> p s c", c=C2),
                in_=on[:, 0:ng * HC].rearrange(
                    "p (s h c) -> p s c h", h=H, c=C),
                axis=mybir.AxisListType.X)
            stage = ep.tile([P, SG * C2], F32, tag="stage2")
            nc.vector.tensor_tensor(out=stage[:, 0:ng * C2],
                                    in0=red[:, 0:ng * C2],
                                    in1=bbc[:, 0:ng * C2],
                                    op=mybir.AluOpType.add)
            store_cb(s0, ng, stage, None)
        group = []
        accbuf = None

    for s in range(slots):
        ta, tb = plan["slot_tiles"][s]
        T_s = ta + tb
        g0 = plan["tile_off"][s]
        gb = gp.tile([P, Tmax, ROW], BF, tag="gb")
        for (grp, toff, nt) in calls_by_slot[s]:
            src_tab = T_dram[0:min(SPLIT, NPAD), :] if grp == 0 \
                else T_dram[SPLIT:NPAD, :]
            nc.gpsimd.dma_gather(
                out_ap=gb[:, toff:toff + nt, :],
                in_ap=src_tab,
                idxs_ap=gidx[:, (g0 + toff) * 8:(g0 + toff + nt) * 8],
                num_idxs=nt * P,
                num_idxs_reg=nt * P,
                elem_size=ROW,
            )
        # fp8 one-hot tables for the whole slot (plain HWDGE loads)
        stb = stp.tile([P, Tmax * P], E4, tag="stb")
        nc.sync.dma_start(out=stb[:, 0:T_s * P],
                          in_=st8_d[:, g0 * P:(g0 + T_s) * P])
        sb = stp.tile([P, Tmax * P], E4, tag="sb")
        nc.sync.dma_start(out=sb[:, 0:T_s * P],
                          in_=s8_d[:, g0 * P:(g0 + T_s) * P])
        adb = sp.tile([P, H], BF, tag="adb")
        nc.vector.tensor_copy(out=adb[:], in_=ado[:, s * H:(s + 1) * H])
        acc = ap.tile([P, W], F32, tag="acc")

        for c0 in range(0, T_s, G):
            nt = min(G, T_s - c0)
            uEp = pp.tile([P, G * H], F32, tag="uEp")
            for i in range(nt):
                nc.tensor.matmul(
                    out=uEp[:, i * H:(i + 1) * H],
                    lhsT=stb[:, (c0 + i) * P:(c0 + i + 1) * P],
                    rhs=adb[:], start=True, stop=True)
            u = sp.tile([P, G * H], F32, tag="u")
            nc.vector.tensor_tensor(
                out=u[:, 0:nt * H].rearrange("p (t h) -> p t h", h=H),
                in0=uEp[:, 0:nt * H].rearrange("p (t h) -> p t h", h=H),
                in1=gb[:, c0:c0 + nt, HC:HC + 2 * H].bitcast(F32),
                op=mybir.AluOpType.add)
            lr = sp.tile([P, G * H], F32, tag="lr")
            nc.vector.scalar_tensor_tensor(
                out=lr[:, 0:nt * H], in0=u[:, 0:nt * H], scalar=NEG_SLOPE,
                in1=u[:, 0:nt * H],
                op0=mybir.AluOpType.mult, op1=mybir.AluOpType.max)
            Mp = sp.tile([P, G, W], BF, tag="Mp")
            nc.scalar.activation(
                out=Mp[:, 0:nt, HC:HC + H],
                in_=lr[:, 0:nt * H].rearrange("p (t h) -> p t h", h=H),
                func=mybir.ActivationFunctionType.Exp)
            pexp = sp.tile([P, G * HC], BF, tag="pexp")
            nc.scalar.activation(
                out=pexp[:, 0:nt * HC].rearrange(
                    "p (t h c) -> p t h c", h=H, c=C),
                in_=lr[:, 0:nt * H].rearrange(
                    "p (t h) -> p t h", h=H).to_broadcast([P, nt, H, C]),
                func=mybir.ActivationFunctionType.Exp)
            nc.vector.tensor_tensor(
                out=Mp[:, 0:nt, 0:HC],
                in0=gb[:, c0:c0 + nt, 0:HC],
                in1=pexp[:, 0:nt * HC].rearrange("p (t m) -> p t m", m=HC),
                op=mybir.AluOpType.mult)
            for i in range(nt):
                t = c0 + i
                nc.tensor.matmul(out=acc[:], lhsT=sb[:, (c0 + i) * P:
                                                     (c0 + i + 1) * P],
                                 rhs=Mp[:, i, :],
                                 start=(t == 0), stop=(t == T_s - 1))
        # stash acc into the epilogue staging buffer
        gi = len(group)
        if gi == 0:
            accbuf = ep.tile([P, SG * W], F32, tag="accbuf")
        nc.vector.tensor_copy(out=accbuf[:, gi * W:(gi + 1) * W], in_=acc[:])
        group.append((s, acc))
        if len(group) == SG or s == slots - 1:
            flush_group()

    pools = (cp, gp, stp, sp, pp, ap, ep)
    return pools


def build_A1(cfg, plan):
    """Layer-1 edge phase (h1' kept in SBUF) + distributed phase-0 of
    layer 2 (-> t2s = [h2 | as2 | ad2] own slice)."""
    NPC, NPAD, D1, D2 = cfg["NPC"], cfg["NPAD"], cfg["D1"], cfg["D2"]
    H1, H2, ROW1 = cfg["H1"], cfg["H2"], cfg["ROW1"]
    slots = cfg["SLOTS"]
    OC2 = D2 + 4 * H2
    nc = bacc.Bacc("TRN2", target_bir_lowering=False, debug=False)
    T1 = nc.declare_dram_parameter("T1", [NPAD, ROW1], BF, isOutput=False)
    ad1 = nc.declare_dram_parameter("ad1", [P, slots * H1], F32,
                                    isOutput=False)
    gx = nc.declare_dram_parameter("gidx", [P, plan["Lg"]], I16,
                                   isOutput=False)
    s8 = nc.declare_dram_parameter("sf8", [P, plan["ttot"] * P], E4,
                                   isOutput=False)
    st8 = nc.declare_dram_parameter("stf8", [P, plan["ttot"] * P], E4,
                                    isOutput=False)
    b1 = nc.declare_dram_parameter("b1bc", [P, SG * D1], F32, isOutput=False)
    W2 = nc.declare_dram_parameter("W2", [D1, D2], BF, isOutput=False)
    WA2 = nc.declare_dram_parameter("WA2", [D1, 2 * H2], BF, isOutput=False)
    t2s = nc.declare_dram_parameter("t2s", [NPC, OC2], BF, isOutput=True)

    with tile.TileContext(nc) as tc:
        with tc.tile_pool(name="g", bufs=1) as gcp, \
             tc.tile_pool(name="p0w", bufs=3) as wp0, \
             tc.tile_pool(name="p0p", bufs=1, space="PSUM") as pp0:
            identb = gcp.tile([P, P], BF, tag="identb")
            make_identity(nc, identb[:])
            w2 = gcp.tile([D1, D2], BF, tag="w2")
            nc.sync.dma_start(out=w2[:], in_=W2[:])
            wa2 = gcp.tile([D1, 2 * H2], BF, tag="wa2")
            nc.sync.dma_start(out=wa2[:], in_=WA2[:])
            h1stage = gcp.tile([P, slots * D1], BF, tag="h1stage")
            t2stage = gcp.tile([P, slots * OC2], BF, tag="t2stage")

            def store1(s0, ng, ex, rl):
                # ELU tail: h1' = rl + ex - 1, straight into h1stage (bf16)
                nc.vector.scalar_tensor_tensor(
                    out=h1stage[:, s0 * D1:(s0 + ng) * D1],
                    in0=ex[:, 0:ng * D1], scalar=-1.0, in1=rl[:, 0:ng * D1],
                    op0=mybir.AluOpType.add, op1=mybir.AluOpType.add)
                # phase-0 of layer 2 for these slots
                for s in range(s0, s0 + ng):
                    hsl = h1stage[:, s * D1:(s + 1) * D1]
                    h1Tp = pp0.tile([P, P], BF, tag="h1Tp")
                    nc.tensor.transpose(out=h1Tp[:], in_=hsl,
                                        identity=identb[:])
                    h1T = wp0.tile([P, P], BF, tag="h1T")
                    nc.vector.tensor_copy(out=h1T[:], in_=h1Tp[:])
                    h2p = pp0.tile([P, D2], F32, tag="h2p")
                    nc.tensor.matmul(out=h2p[:], lhsT=h1T[:], rhs=w2[:],
                                     start=True, stop=True)
                    nc.scalar.copy(
                        out=t2stage[:, s * OC2:s * OC2 + D2], in_=h2p[:])
                    aap = pp0.tile([P, 2 * H2], F32, tag="aap")
                    nc.tensor.matmul(out=aap[:], lhsT=h1T[:], rhs=wa2[:],
                                     start=True, stop=True)
                    nc.scalar.copy(
                        out=t2stage[:, s * OC2 + D2:(s + 1) * OC2]
                        .bitcast(F32),
                        in_=aap[:])

            pools = _edge_phase(nc, tc, cfg, plan, 1, T1, ad1, gx,
                                s8, st8, b1, store1)
            nc.sync.dma_start(
                out=t2s.rearrange("(s p) c -> p s c", p=P),
                in_=t2stage[:].rearrange("p (s c) -> p s c", c=OC2))
            for pl in reversed(pools):
                pl.release()
    nc.compile()
    return nc


def build_B1(cfg, plan):
    NPC, NPAD, H2, C2 = cfg["NPC"], cfg["NPAD"], cfg["H2"], cfg["C2"]
    ROW2, slots = cfg["ROW2"], cfg["SLOTS"]
    nc = bacc.Bacc("TRN2", target_bir_lowering=False, debug=False)
    T2 = nc.declare_dram_parameter("T2", [NPAD, ROW2], BF, isOutput=False)
    ad2 = nc.declare_dram_parameter("ad2", [P, slots * H2], F32,
                                    isOutput=False)
    gx = nc.declare_dram_parameter("gidx", [P, plan["Lg"]], I16,
                                   isOutput=False)
    s8 = nc.declare_dram_parameter("sf8", [P, plan["ttot"] * P], E4,
                                   isOutput=False)
    st8 = nc.declare_dram_parameter("stf8", [P, plan["ttot"] * P], E4,
                                    isOutput=False)
    b2 = nc.declare_dram_parameter("b2bc", [P, SG * C2], F32, isOutput=False)
    out2 = nc.declare_dram_parameter("out2", [NPC, C2], F32, isOutput=True)
    with tile.TileContext(nc) as tc:
        with tc.tile_pool(name="g2", bufs=1) as gcp:
            ostage = gcp.tile([P, slots * C2], F32, tag="ostage")

            def store2(s0, ng, stage, _):
                nc.vector.tensor_copy(
                    out=ostage[:, s0 * C2:(s0 + ng) * C2],
                    in_=stage[:, 0:ng * C2])

            pools = _edge_phase(nc, tc, cfg, plan, 2, T2, ad2, gx,
                                s8, st8, b2, store2)
            nc.sync.dma_start(
                out=out2.rearrange("(s p) c -> p s c", p=P),
                in_=ostage[:].rearrange("p (s c) -> p s c", c=C2))
            for pl in reversed(pools):
                pl.release()
    nc.compile()
    return nc


# ---------------------------------------------------------------------------
# Host orchestration
# ---------------------------------------------------------------------------

def _block_diag_att(att):
    """att [H, C] -> [H*C, H] block diagonal."""
    H, C = att.shape
    out = np.zeros((H * C, H), np.float32)
    for h in range(H):
        out[h * C:(h + 1) * C, h] = att[h]
    return out


_CACHE = {}


def _get_programs(cfg, plan):
    key = (cfg["N"], cfg["E"], tuple(plan["slot_tiles"]), plan["ncalls"])
    if key not in _CACHE:
        _CACHE[key] = (build_A0(cfg), build_A1(cfg, plan),
                       build_B1(cfg, plan))
    return _CACHE[key]


def _run(nc, in_maps, **kw):
    res = run_bass_kernel_spmd(nc, in_maps, list(range(NCORES)), **kw)
    return res


def _run_timed(nc, in_maps, n_iters=3):
    """Like bass2jax.run_bass_via_pjrt but with device-resident inputs and
    repeated timed executes (min wall over n_iters after warmup)."""
    import time
    import jax
    from jax.sharding import Mesh, PartitionSpec, NamedSharding
    from jax.experimental.shard_map import shard_map
    from concourse.bass2jax import _bass_exec_p, partition_id_tensor, \
        install_neuronx_cc_hook

    install_neuronx_cc_hook()
    n_cores = len(in_maps)
    partition_name = nc.partition_id_tensor.name if nc.partition_id_tensor \
        else None
    in_names, out_names, out_avals, zero_outs = [], [], [], []
    for alloc in nc.m.functions[0].allocations:
        if not isinstance(alloc, mybir.MemoryLocationSet):
            continue
        name = alloc.memorylocations[0].name
        if alloc.kind == "ExternalInput":
            if name != partition_name:
                in_names.append(name)
        elif alloc.kind == "ExternalOutput":
            shape = tuple(alloc.tensor_shape)
            dtype = mybir.dt.np(alloc.dtype)
            out_names.append(name)
            out_avals.append(jax.core.ShapedArray(shape, dtype))
            zero_outs.append(np.zeros(shape, dtype))
    n_params = len(in_names)
    n_outs = len(out_avals)
    in_names_all = in_names + out_names
    if partition_name is not None:
        in_names_all = in_names_all + [partition_name]

    def _body(*args):
        operands = list(args)
        if partition_name is not None:
            operands.append(partition_id_tensor())
        return tuple(_bass_exec_p.bind(
            *operands, out_avals=tuple(out_avals),
            in_names=tuple(in_names_all), out_names=tuple(out_names),
            lowering_input_output_aliases=(),
            sim_require_finite=True, sim_require_nnan=True, nc=nc))

    devices = jax.devices()[:n_cores]
    mesh = Mesh(np.asarray(devices), ("core",))
    spec = PartitionSpec("core")
    sharded = jax.jit(
        shard_map(_body, mesh=mesh, in_specs=(spec,) * (n_params + n_outs),
                  out_specs=(spec,) * n_outs, check_rep=False),
        keep_unused=True)
    sh = NamedSharding(mesh, spec)
    dev_in = [
        jax.device_put(
            np.concatenate([np.asarray(in_maps[c][nm]) for c in
                            range(n_cores)], axis=0), sh)
        for nm in in_names
    ]
    dev_zero = [
        jax.device_put(
            np.zeros((n_cores * z.shape[0], *z.shape[1:]), z.dtype), sh)
        for z in zero_outs
    ]
    out = sharded(*dev_in, *dev_zero)  # warmup + compile
    jax.block_until_ready(out)
    wall = []
    for _ in range(n_iters):
        t0 = time.perf_counter()
        o = sharded(*dev_in, *dev_zero)
        jax.block_until_ready(o)
        wall.append(time.perf_counter() - t0)
    results = [
        {nm: np.asarray(out[i]).reshape(n_cores, *out_avals[i].shape)[c]
         for i, nm in enumerate(out_names)}
        for c in range(n_cores)
    ]

    class R:
        pass
    r = R()
    r.results = results
    r.exec_time_ns = int(min(wall) * 1e9)
    r.wall_all = wall
    return r


def kernel(x, edge_index, W1, att_src1, att_dst1, b1, W2, att_src2,
           att_dst2, b2, _collect_times=None, _cfg_override=None,
           _runner=None):
    cfg = _cfg_override or CFG
    N, NPC, NPAD = cfg["N"], cfg["NPC"], cfg["NPAD"]
    D1, D2, H1, H2, C2 = cfg["D1"], cfg["D2"], cfg["H1"], cfg["H2"], cfg["C2"]
    ROW1, ROW2, slots = cfg["ROW1"], cfg["ROW2"], cfg["SLOTS"]

    x = np.asarray(x, np.float32)
    ei = np.asarray(edge_index)
    loops = np.arange(N, dtype=ei.dtype)
    src_n = np.concatenate([ei[0], loops])
    dst_n = np.concatenate([ei[1], loops])

    plan, edata = build_edge_plan(cfg, src_n, dst_n)
    ncA0, ncA1, ncB1 = _get_programs(cfg, plan)
    if _runner is not None:
        run = _runner
    elif _collect_times is not None:
        run = _run_timed
    else:
        run = _run

    # ---- launch A0: per-node h1/as1/ad1 ----
    xpad = np.zeros((NCORES * NPC, cfg["IN"]), np.float32)
    xpad[:N] = x
    W1f = np.asarray(W1, np.float32)
    AA1 = np.concatenate([_block_diag_att(np.asarray(att_src1, np.float32)),
                          _block_diag_att(np.asarray(att_dst1, np.float32))],
                         axis=1)
    WA1 = (W1f @ AA1).astype(BF16)
    in_maps = [
        dict(xs=xpad[c * NPC:(c + 1) * NPC],
             W1=W1f.astype(BF16), WA1=WA1)
        for c in range(NCORES)
    ]
    resA0 = run(ncA0, in_maps)
    t1s = [resA0.results[c]["t1s"] for c in range(NCORES)]
    if _collect_times is not None:
        _collect_times.append(("A0", resA0.exec_time_ns))

    # assemble replicated T1 [NPAD, ROW1]
    OC1 = D1 + 4 * H1
    T1 = np.zeros((NPAD, ROW1), BF16)
    full = np.concatenate(t1s, axis=0)  # [8*NPC, OC1]
    T1[:, :OC1] = full[:NPAD]
    ad1full = np.ascontiguousarray(full[:, D1 + 2 * H1:OC1]).view(
        np.float32)[:, H1:]  # [8*NPC, H1] a_dst per global node
    assert ad1full.shape[1] == H1
    blk_of = plan["blk_of"]
    ad1own = np.zeros((NCORES, P, slots * H1), np.float32)
    for c in range(NCORES):
        for s in range(slots):
            b = blk_of[c, s]
            if b >= 0:
                ad1own[c, :, s * H1:(s + 1) * H1] = \
                    ad1full[b * P:(b + 1) * P]

    # ---- launch A1: layer-1 edges + phase0 of layer 2 ----
    W2f = np.asarray(W2, np.float32)
    AA2 = np.concatenate([_block_diag_att(np.asarray(att_src2, np.float32)),
                          _block_diag_att(np.asarray(att_dst2, np.float32))],
                         axis=1)
    WA2 = (W2f @ AA2).astype(BF16)
    b1bc = np.tile(np.asarray(b1, np.float32)[None, :], (P, SG))
    in_maps = [
        dict(T1=T1, ad1=np.ascontiguousarray(ad1own[c]),
             gidx=edata["gidx"][c], sf8=edata["sf8"][c],
             stf8=edata["stf8"][c], b1bc=b1bc,
             W2=W2f.astype(BF16), WA2=WA2)
        for c in range(NCORES)
    ]
    resA1 = run(ncA1, in_maps)
    t2s = [resA1.results[c]["t2s"] for c in range(NCORES)]
    if _collect_times is not None:
        _collect_times.append(("A1", resA1.exec_time_ns))

    OC2 = D2 + 4 * H2
    T2 = np.zeros((NPAD, ROW2), BF16)
    for c in range(NCORES):
        for s in range(slots):
            b = blk_of[c, s]
            if b >= 0:
                T2[b * P:(b + 1) * P, :OC2] = t2s[c][s * P:(s + 1) * P]
    ad2full = np.ascontiguousarray(T2[:, D2 + 2 * H2:OC2]).view(
        np.float32)[:, H2:]
    ad2own = np.zeros((NCORES, P, slots * H2), np.float32)
    for c in range(NCORES):
        for s in range(slots):
            b = blk_of[c, s]
            if b >= 0:
                ad2own[c, :, s * H2:(s + 1) * H2] = \
                    ad2full[b * P:(b + 1) * P]

    # ---- launch B1: layer-2 edges ----
    b2bc = np.tile(np.asarray(b2, np.float32)[None, :], (P, SG))
    in_maps = [
        dict(T2=T2, ad2=np.ascontiguousarray(ad2own[c]),
             gidx=edata["gidx"][c], sf8=edata["sf8"][c],
             stf8=edata["stf8"][c], b2bc=b2bc)
        for c in range(NCORES)
    ]
    resB1 = run(ncB1, in_maps)
    if _collect_times is not None:
        _collect_times.append(("B1", resB1.exec_time_ns))
    out = np.zeros((NPAD, C2), np.float32)
    for c in range(NCORES):
        o = resB1.results[c]["out2"]
        for s in range(slots):
            b = blk_of[c, s]
            if b >= 0:
                out[b * P:(b + 1) * P] = o[s * P:(s + 1) * P]
    return np.asarray(out[:N], np.float32)
